# revision 19
# baseline (speedup 1.0000x reference)
"""Trainium2 Bass kernel for nn_ContrastiveLoss (bs=128, nw=80, nf=64, d=768).

Strategy (v2: word-sharded fp8 DoubleRow sweep)
-----------------------------------------------
All four similarity paths of the module are slices of ONE augmented dot-product
tensor  G[t, wa, v, fa] = aug_w[t, wa] . aug_f[v, fa]  where
  aug_w = [word_features (80), sentence_output (1)]
  aug_f = [frame_features (64), traj_output (1)]
With TAU = 0.01 every softmax-weighted pooling is within tau*ln(n) <= 0.05 of
a plain max (validated: end-to-end rel err ~1e-4), so the whole fine-grained
path collapses to max-reductions fused onto the matmul's PSUM output.

v2 sharding: the 80 WORDS are split 10-per-core across 8 cores; every core
holds ALL 128 videos. This gives each stationary [d,t] block 8320 moving
columns of reuse (vs 1040 under video-sharding), eliminating ~1000 tiny
8-column matmuls, and makes the cross-core combine an elementwise MAX:
  fw[t,v] = max_cores max_{m in core, fa} S     (AllReduce-max)
  vw[t,v] = max_cores max_{m in core} A         (same AllReduce)
The sentence row (sf = max_fa B, ts = sent.traj) is computed per-core for its
own 16 videos in bf16 and AllGathered (tiny, fully hidden).

The fine sweep runs in fp8 (float8e4) with MatmulPerfMode.DoubleRow: two
128-deep k-chunks per instruction at 2x FLOP rate. fp8 operand noise on a
768-deep dot is ~1.4 absolute on values whose pairwise gaps at the max are
~6+; measured end-to-end loss error is ~1e-4 (tolerance 2e-2).

Per half (64 videos) each core does 10 words x 8 blocks x 3 k-pair matmuls of
[128,512]; the fa-max reduce is split between the Vector engine (direct f32
PSUM reduce) and the Scalar engine (PSUM->SBUF bf16 convert, then a 4x-rate
bf16 DVE reduce) to balance engine load. The half-0 AllReduce(max) overlaps
the half-1 sweep; after the half-1 AllReduce only ~4us of cross-entropy
remains (both ACT tables are pre-warmed during the DMA ramp).
"""

import os
import sys
from contextlib import ExitStack

import numpy as np
import ml_dtypes

_REPO = "/opt/trn_rl_repo"
if os.path.isdir(_REPO) and _REPO not in sys.path:
    sys.path.insert(0, _REPO)

BS, NW, NF, D = 128, 80, 64, 768
N_CORES = 8
MPC = NW // N_CORES          # words per core = 10
NJ = 3                       # k-pairs (768 = 3 * 256)
FINE = BS * NF               # fine moving cols per k-plane = 8192
FREE8 = FINE + BS            # + 128 traj cols = 8320
TAU = 0.01
NEG = -3.0e38

_CACHE = {}


def _build_nc(n_cores=N_CORES):
    """Build + compile the SPMD per-core program (identical on all cores)."""
    from concourse import bacc, mybir, tile

    F32 = mybir.dt.float32
    BF16 = mybir.dt.bfloat16
    FP8 = mybir.dt.float8e4
    AX = mybir.AxisListType.X
    ALU = mybir.AluOpType
    ACT = mybir.ActivationFunctionType
    DR = mybir.MatmulPerfMode.DoubleRow

    nc = bacc.Bacc(
        "TRN2", target_bir_lowering=False, debug=False, num_devices=n_cores
    )
    # DoubleRow operands are plane-major [p, i, cols] (i = which 128-deep
    # k-chunk of the 256 pair); walrus requires unit-stride columns.
    #   wf8 cols: m*128 + t ; ff8 cols: v*64+fa (fine, vl-major) | 8192+v (traj)
    wf8_d = nc.dram_tensor("wf8", [NJ, 128, 2, MPC * 128], FP8, kind="ExternalInput")
    ff8_d = nc.dram_tensor("ff8", [NJ, 128, 2, FREE8], FP8, kind="ExternalInput")
    sent_d = nc.dram_tensor("sent", [6, 128, 128], BF16, kind="ExternalInput")
    ffv_d = nc.dram_tensor("ffv", [6, 128, 1040], BF16, kind="ExternalInput")
    eye_d = nc.dram_tensor("eye", [128, 128], F32, kind="ExternalInput")
    loss_d = nc.dram_tensor("loss", [1, 1], F32, kind="ExternalOutput")
    sim_d = nc.dram_tensor("sim", [BS, BS], F32, kind="ExternalOutput")

    with tile.TileContext(nc) as tc, ExitStack() as ctx:
        cpool = ctx.enter_context(tc.tile_pool(name="const", bufs=1))
        psA_pool = ctx.enter_context(tc.tile_pool(name="psA", bufs=6, space="PSUM"))
        psb_pool = ctx.enter_context(tc.tile_pool(name="psb", bufs=1, space="PSUM"))
        ps1_pool = ctx.enter_context(tc.tile_pool(name="ps1", bufs=1, space="PSUM"))
        tmp_pool = ctx.enter_context(tc.tile_pool(name="tmp", bufs=3))
        tmpb_pool = ctx.enter_context(tc.tile_pool(name="tmpb", bufs=4))
        dram = ctx.enter_context(tc.tile_pool(name="dram", bufs=1, space="DRAM"))

        # ---- DMA: half-0 j0 operands first so the PE starts ~4us in ------
        ff_sb = [cpool.tile([128, 2, FREE8], FP8, name=f"ff{j}") for j in range(NJ)]
        wf_sb = [cpool.tile([128, 2, MPC * 128], FP8, name=f"wf{j}") for j in range(NJ)]

        def ff_slice(j, c0, c1):  # cols [c0,c1) of both i-planes
            return ff_sb[j][:, :, c0:c1]

        def ff_dram_slice(j, c0, c1):
            return ff8_d.ap()[j][:, :, c0:c1]

        # The first block-pair needs (wf8, ff8 cols 0:1024) for ALL 3 k-pairs;
        # ship exactly that first so the PE starts ~4us in, then the rest of
        # half 0, then the sentence operands, then half 1.
        for j in range(NJ):
            nc.sync.dma_start(ff_slice(j, 0, 1024), ff_dram_slice(j, 0, 1024))
            nc.sync.dma_start(wf_sb[j][:], wf8_d.ap()[j])
        for j in range(NJ):
            nc.sync.dma_start(ff_slice(j, 1024, 4096), ff_dram_slice(j, 1024, 4096))
            nc.sync.dma_start(ff_slice(j, FINE, FINE + 64), ff_dram_slice(j, FINE, FINE + 64))
        sent_sb = [cpool.tile([128, 128], BF16, name=f"se{k}") for k in range(6)]
        ffv_sb = [cpool.tile([128, 1040], BF16, name=f"fv{k}") for k in range(6)]
        for k in range(6):
            nc.sync.dma_start(sent_sb[k][:], sent_d.ap()[k])
            nc.sync.dma_start(ffv_sb[k][:], ffv_d.ap()[k])
        eye_sb = cpool.tile([128, 128], F32, name="eye_sb")
        nc.gpsimd.dma_start(eye_sb[:], eye_d.ap())
        for j in range(NJ):  # half-1 fine + traj
            nc.sync.dma_start(ff_slice(j, 4096, FINE), ff_dram_slice(j, 4096, FINE))
            nc.sync.dma_start(
                ff_slice(j, FINE + 64, FREE8), ff_dram_slice(j, FINE + 64, FREE8)
            )

        # ---- accumulators + collective buffers ---------------------------
        fwvw = [cpool.tile([128, 128], F32, name=f"fwvw{h}") for h in range(2)]
        nc.vector.memset(fwvw[0][:], NEG)
        nc.vector.memset(fwvw[1][:], NEG)
        s2own = cpool.tile([128, 16], F32, name="s2own")

        ar_in = [dram.tile([128, 128], F32, name=f"ar_in{h}") for h in range(2)]
        ar_out = [
            dram.tile([128, 128], F32, name=f"ar_out{h}", addr_space="Shared")
            for h in range(2)
        ]
        ag_in = dram.tile([128, 16], F32, name="ag_in")
        ag_out = dram.tile([n_cores, 128, 16], F32, name="ag_out", addr_space="Shared")

        # pre-warm both ACT tables (Exp, Ln) so no table load lands in the
        # post-collective tail
        warm = cpool.tile([1, 1], F32, name="warm")
        nc.gpsimd.memset(warm[:], 1.0)
        warm2 = cpool.tile([1, 1], F32, name="warm2")
        nc.scalar.activation(warm2[:], warm[:], ACT.Exp)
        nc.scalar.activation(warm2[:], warm[:], ACT.Ln)

        def wf_ap(j, m):  # stationary [128, 2, 128] for word m, k-pair j
            return wf_sb[j][:, :, m * 128 : (m + 1) * 128]

        DIRECT = (0, 3, 6)  # blocks reduced straight from PSUM on the DVE

        def sentence_sweep():
            # bf16 sentence x (own 16 videos + their traj), AllGather s2.
            # Emitted mid-half-0 so the AG clears the CC stream long before
            # the AllReduces need it.
            psS = [
                psA_pool.tile([128, 512], F32, tag="psA", name=f"psS{q}")
                for q in range(2)
            ]
            psBs = psb_pool.tile([128, 64], F32, tag="psB", name="psBs")
            for k in range(6):
                for q in range(2):
                    nc.tensor.matmul(
                        psS[q][:], lhsT=sent_sb[k][:],
                        rhs=ffv_sb[k][:, q * 512 : (q + 1) * 512],
                        start=(k == 0), stop=(k == 5),
                    )
                nc.tensor.matmul(
                    psBs[:, 0:16], lhsT=sent_sb[k][:], rhs=ffv_sb[k][:, 1024:1040],
                    start=(k == 0), stop=(k == 5),
                )
            for q in range(2):
                nc.vector.reduce_max(
                    s2own[:, 8 * q : 8 * q + 8],
                    psS[q][:].rearrange("p (vl fa) -> p vl fa", vl=8),
                    axis=AX,
                )
            nc.vector.tensor_add(s2own[:], s2own[:], psBs[:, 0:16])
            nc.sync.dma_start(ag_in[:], s2own[:])
            nc.gpsimd.collective_compute(
                "AllGather", ALU.bypass, replica_groups=[list(range(n_cores))],
                ins=[ag_in[:].opt()], outs=[ag_out[:].opt()],
            )

        def fine_half(h):
            c0 = h * 4096
            for m in range(MPC):
                if h == 0 and m == 5:
                    sentence_sweep()
                fa_red = tmp_pool.tile([128, 64], F32, tag="fa_red")
                for pair in range(4):
                    ps = [
                        psA_pool.tile([128, 512], F32, tag="psA", name=f"psA_{q}")
                        for q in range(2)
                    ]
                    for j in range(NJ):
                        for q in range(2):
                            b = 2 * pair + q
                            nc.tensor.matmul(
                                ps[q][:],
                                lhsT=wf_ap(j, m),
                                rhs=ff_slice(j, c0 + b * 512, c0 + (b + 1) * 512),
                                start=(j == 0),
                                stop=(j == NJ - 1),
                                perf_mode=DR,
                            )
                    for q in range(2):
                        b = 2 * pair + q
                        pv = ps[q][:].rearrange("p (vl fa) -> p vl fa", vl=8)
                        if b in DIRECT:
                            nc.vector.reduce_max(fa_red[:, 8 * b : 8 * b + 8], pv, axis=AX)
                        else:
                            sc = tmpb_pool.tile([128, 512], BF16, tag="scb")
                            nc.scalar.activation(sc[:], ps[q][:], ACT.Copy)
                            nc.vector.reduce_max(
                                fa_red[:, 8 * b : 8 * b + 8],
                                sc[:].rearrange("p (vl fa) -> p vl fa", vl=8),
                                axis=AX,
                            )
                # traj cols for this half: A[t, v] partial
                psB = psb_pool.tile([128, 64], F32, tag="psB")
                tc0 = FINE + 64 * h
                for j in range(NJ):
                    nc.tensor.matmul(
                        psB[:],
                        lhsT=wf_ap(j, m),
                        rhs=ff_slice(j, tc0, tc0 + 64),
                        start=(j == 0),
                        stop=(j == NJ - 1),
                        perf_mode=DR,
                    )
                nc.vector.tensor_max(fwvw[h][:, 0:64], fwvw[h][:, 0:64], fa_red[:])
                nc.vector.tensor_max(fwvw[h][:, 64:128], fwvw[h][:, 64:128], psB[:])

        # ---- half 0 sweep, AllReduce(max) #0 ------------------------------
        fine_half(0)
        nc.sync.dma_start(ar_in[0][:], fwvw[0][:])
        nc.gpsimd.collective_compute(
            "AllReduce", ALU.max, replica_groups=[list(range(n_cores))],
            ins=[ar_in[0][:].opt()], outs=[ar_out[0][:].opt()],
        )

        # ---- half 1 sweep, AllReduce(max) #1 ------------------------------
        fine_half(1)
        nc.sync.dma_start(ar_in[1][:], fwvw[1][:])
        nc.gpsimd.collective_compute(
            "AllReduce", ALU.max, replica_groups=[list(range(n_cores))],
            ins=[ar_in[1][:].opt()], outs=[ar_out[1][:].opt()],
        )

        # ---- cross-entropy ------------------------------------------------
        # sim[:, 64h:64h+64] = (fw + vw + s2)/4 ; CE-0 runs under AR#1's
        # shadow, CE-1 + the tiny merge after it.
        s2full = cpool.tile([128, 128], F32, name="s2full")
        nc.sync.dma_start(
            s2full[:].rearrange("p (r c) -> p r c", r=n_cores),
            ag_out[:].rearrange("r p c -> p r c"),
        )
        ones = cpool.tile([128, 1], F32, name="ones")
        nc.gpsimd.memset(ones[:], 1.0)

        mxr = cpool.tile([128, 2], F32, name="mxr")
        nmxr = cpool.tile([128, 2], F32, name="nmxr")
        er = cpool.tile([128, 2], F32, name="er")
        dgh = cpool.tile([128, 2], F32, name="dgh")
        mxc = cpool.tile([64, 2], F32, name="mxc")
        nmxc = cpool.tile([64, 2], F32, name="nmxc")
        ec = cpool.tile([64, 2], F32, name="ec")
        lec = cpool.tile([64, 2], F32, name="lec")
        sL = [cpool.tile([128, 64], F32, name=f"sL{h}") for h in range(2)]

        for h in range(2):
            hh = slice(h, h + 1)
            arf = cpool.tile([128, 128], F32, name=f"arf{h}")
            nc.sync.dma_start(arf[:], ar_out[h][:])
            nc.vector.tensor_add(sL[h][:], arf[:, 0:64], arf[:, 64:128])
            nc.vector.tensor_add(sL[h][:], sL[h][:], s2full[:, 64 * h : 64 * h + 64])
            nc.vector.tensor_scalar_mul(sL[h][:], sL[h][:], 0.25)
            nc.gpsimd.dma_start(sim_d.ap()[:, 64 * h : 64 * h + 64], sL[h][:])

            nc.vector.reduce_max(mxr[:, hh], sL[h][:], axis=AX)
            nc.vector.tensor_scalar_mul(nmxr[:, hh], mxr[:, hh], -1.0)
            scr = tmp_pool.tile([128, 64], F32, tag="scr")
            nc.scalar.activation(
                scr[:], sL[h][:], ACT.Exp,
                bias=nmxr[:, hh], scale=1.0, accum_out=er[:, hh],
            )
            scr2 = tmp_pool.tile([128, 64], F32, tag="scr")
            nc.vector.tensor_mul(scr2[:], sL[h][:], eye_sb[:, 64 * h : 64 * h + 64])
            nc.vector.reduce_sum(dgh[:, hh], scr2[:], axis=AX)
            # column stats via 32x32 DVE block transposes
            sLT = cpool.tile([64, 128], F32, name=f"sLT{h}")
            for bi in range(4):
                for bj in range(2):
                    nc.vector.transpose(
                        sLT[32 * bj : 32 * bj + 32, 32 * bi : 32 * bi + 32],
                        sL[h][32 * bi : 32 * bi + 32, 32 * bj : 32 * bj + 32],
                    )
            nc.vector.reduce_max(mxc[:, hh], sLT[:], axis=AX)
            nc.vector.tensor_scalar_mul(nmxc[:, hh], mxc[:, hh], -1.0)
            scr3 = tmp_pool.tile([64, 128], F32, tag="scrT")
            nc.scalar.activation(
                scr3[:], sLT[:], ACT.Exp,
                bias=nmxc[:, hh], scale=1.0, accum_out=ec[:, hh],
            )

        # merge row stats across halves; assemble loss
        Mrow = cpool.tile([128, 1], F32, name="Mrow")
        nMrow = cpool.tile([128, 1], F32, name="nMrow")
        dsc = cpool.tile([128, 2], F32, name="dsc")
        ew = cpool.tile([128, 2], F32, name="ew")
        es = cpool.tile([128, 1], F32, name="es")
        lser = cpool.tile([128, 1], F32, name="lser")
        nc.vector.tensor_max(Mrow[:], mxr[:, 0:1], mxr[:, 1:2])
        nc.vector.tensor_scalar_mul(nMrow[:], Mrow[:], -1.0)
        nc.scalar.activation(dsc[:], mxr[:], ACT.Exp, bias=nMrow[:], scale=1.0)
        nc.vector.tensor_mul(ew[:], er[:], dsc[:])
        nc.vector.reduce_sum(es[:], ew[:], axis=AX)
        nc.scalar.activation(lec[:], ec[:], ACT.Ln)
        nc.scalar.activation(lser[:], es[:], ACT.Ln)

        dsum = cpool.tile([128, 1], F32, name="dsum")
        rv = cpool.tile([128, 1], F32, name="rv")
        nc.vector.reduce_sum(dsum[:], dgh[:], axis=AX)
        nc.vector.scalar_tensor_tensor(
            out=rv[:], in0=dsum[:], scalar=-2.0, in1=Mrow[:],
            op0=ALU.mult, op1=ALU.add,
        )
        nc.vector.tensor_add(rv[:], rv[:], lser[:])
        cv = cpool.tile([64, 1], F32, name="cv")
        cvb = cpool.tile([64, 1], F32, name="cvb")
        nc.vector.reduce_sum(cv[:], mxc[:], axis=AX)
        nc.vector.reduce_sum(cvb[:], lec[:], axis=AX)
        nc.vector.tensor_add(cv[:], cv[:], cvb[:])

        ps1 = ps1_pool.tile([1, 1], F32, tag="ps1")
        nc.tensor.matmul(ps1[:], lhsT=rv[:], rhs=ones[:], start=True, stop=False)
        nc.tensor.matmul(ps1[:], lhsT=cv[:], rhs=ones[0:64, :], start=False, stop=True)
        lossv = cpool.tile([1, 1], F32, name="lossv")
        nc.vector.tensor_scalar_mul(lossv[:], ps1[:], 1.0 / (2.0 * BS))
        nc.sync.dma_start(loss_d.ap(), lossv[:])

    nc.compile()
    return nc


def _prep_in_maps(wf, ff, so, to, n_cores=N_CORES):
    """Host-side: build per-core operand arrays in matmul layout."""
    fp8 = ml_dtypes.float8_e4m3
    bf = ml_dtypes.bfloat16

    # stationary: wf8[j, p, i, (m, t)] = wf[t, wa=10c+m, d=256j+128i+p]
    W = np.ascontiguousarray(wf.transpose(2, 1, 0)).astype(fp8)   # [d, wa, t]
    W = W.reshape(NJ, 2, 128, NW, BS)                             # [j, i, p, wa, t]

    # moving: ff8[j, p, i, col]; col = v*64+fa | 8192+v
    F = np.ascontiguousarray(ff.transpose(2, 0, 1)).astype(fp8)   # [d, v, fa]
    F = F.reshape(NJ, 2, 128, FINE)                               # [j, i, p, v*64+fa]
    T8 = np.ascontiguousarray(to.T).astype(fp8).reshape(NJ, 2, 128, BS)
    FT = np.concatenate([F, T8], axis=3)                          # [j, i, p, 8320]
    ff8 = np.ascontiguousarray(FT.transpose(0, 2, 1, 3))          # [j, p, i, 8320]

    # sentence stationary: sent[k, p, t] = so[t, 128k+p]
    sent = np.ascontiguousarray(so.T).astype(bf).reshape(6, 128, 128)

    Fb = np.ascontiguousarray(ff.transpose(2, 0, 1)).astype(bf)   # [d, v, fa]
    Fb = Fb.reshape(6, 128, BS, NF)
    Tb = np.ascontiguousarray(to.T).astype(bf).reshape(6, 128, BS)

    eye = np.eye(128, dtype=np.float32)

    in_maps = []
    for c in range(n_cores):
        wc = W[:, :, :, MPC * c : MPC * (c + 1), :]               # [j,i,p,10,t]
        wf8 = np.ascontiguousarray(wc.transpose(0, 2, 1, 3, 4)).reshape(
            NJ, 128, 2, MPC * 128
        )
        ffv = np.concatenate(
            [
                Fb[:, :, 16 * c : 16 * c + 16, :].reshape(6, 128, 1024),
                Tb[:, :, 16 * c : 16 * c + 16],
            ],
            axis=2,
        )
        in_maps.append(
            {"wf8": wf8, "ff8": ff8, "sent": sent,
             "ffv": np.ascontiguousarray(ffv), "eye": eye}
        )
    return in_maps


def _run(in_maps, trace=False):
    from concourse.bass_utils import run_bass_kernel_spmd

    if "nc" not in _CACHE:
        _CACHE["nc"] = _build_nc()
    return run_bass_kernel_spmd(
        _CACHE["nc"], in_maps, core_ids=list(range(N_CORES)), trace=trace
    )


def _numpy_reference(traj_output, frame_features, sentence_output, word_features,
                     global_mat_weight, word_logit_weight, frame_logit_weight,
                     local_mat_weight, frame_mat_weight, word_mat_weight,
                     frame_mat_weight2, word_mat_weight2):
    """Exact f64 fallback (only used if the weight matrices are not identity)."""
    def softmax(x, axis):
        m = np.max(x, axis=axis, keepdims=True)
        e = np.exp(x - m)
        return e / np.sum(e, axis=axis, keepdims=True)

    def log_softmax(x, axis):
        m = np.max(x, axis=axis, keepdims=True)
        return x - m - np.log(np.sum(np.exp(x - m), axis=axis, keepdims=True))

    to = traj_output.astype(np.float64)
    ff = frame_features.astype(np.float64)
    so = sentence_output.astype(np.float64)
    wf = word_features.astype(np.float64)
    G, WL, FL = (global_mat_weight.astype(np.float64),
                 word_logit_weight.astype(np.float64),
                 frame_logit_weight.astype(np.float64))
    LM, FM, WM = (local_mat_weight.astype(np.float64),
                  frame_mat_weight.astype(np.float64),
                  word_mat_weight.astype(np.float64))
    FM2, WM2 = (frame_mat_weight2.astype(np.float64),
                word_mat_weight2.astype(np.float64))

    traj_sent = (so @ G) @ to.T
    A = np.einsum("twd,vd->twv", wf, to)
    sA = softmax(A / TAU, axis=1)
    wA = np.einsum("twv,wu->tuv", sA, WL)
    video_word = np.sum(A * wA, axis=1)
    B = np.einsum("td,vfd->vtf", so, ff)
    sB = softmax(B / TAU, axis=-1)
    sentence_frame = np.sum(B * (sB @ FL), axis=-1).T
    wfl = wf @ LM
    fw = np.zeros((BS, BS))
    for t in range(BS):
        S = np.einsum("wd,vfd->wvf", wfl[t], ff)
        sw = softmax(S / TAU, axis=0)
        word_level = np.sum(np.einsum("wvf,wu->uvf", sw, WM) * S, axis=0)
        sfx = softmax(S / TAU, axis=-1)
        frame_level = np.sum((sfx @ FM) * S, axis=-1)
        smw = softmax(word_level / TAU, axis=-1)
        s2f = np.sum((smw @ FM2) * word_level, axis=-1)
        smf = softmax(frame_level / TAU, axis=0)
        v2w = np.sum(np.einsum("wv,wu->uv", smf, WM2) * frame_level, axis=0)
        fw[t] = (s2f + v2w) / 2.0
    sim = (traj_sent + video_word + sentence_frame + fw) / 4.0

    def ce(m):
        return -np.mean(np.diagonal(log_softmax(m, -1)))

    return np.array((ce(sim) + ce(sim.T)) / 2.0, dtype=np.float32)


def kernel(**inputs):
    wf = np.ascontiguousarray(np.asarray(inputs["word_features"], np.float32))
    ff = np.ascontiguousarray(np.asarray(inputs["frame_features"], np.float32))
    so = np.ascontiguousarray(np.asarray(inputs["sentence_output"], np.float32))
    to = np.ascontiguousarray(np.asarray(inputs["traj_output"], np.float32))

    eye_names = [
        ("global_mat_weight", D), ("word_logit_weight", NW),
        ("frame_logit_weight", NF), ("local_mat_weight", D),
        ("frame_mat_weight", NF), ("word_mat_weight", NW),
        ("frame_mat_weight2", NF), ("word_mat_weight2", NW),
    ]
    for name, n in eye_names:
        w = np.asarray(inputs[name], np.float32)
        if not np.allclose(w, np.eye(n, dtype=np.float32), atol=1e-6):
            return _numpy_reference(**{k: np.asarray(x) for k, x in inputs.items()})

    res = _run(_prep_in_maps(wf, ff, so, to))
    return np.array(res.results[0]["loss"][0, 0], dtype=np.float32)


# revision 25
# speedup vs baseline: 1.0047x; 1.0047x over previous
"""Trainium2 Bass kernel for nn_ContrastiveLoss (bs=128, nw=80, nf=64, d=768).

Strategy (v2: word-sharded fp8 DoubleRow sweep)
-----------------------------------------------
All four similarity paths of the module are slices of ONE augmented dot-product
tensor  G[t, wa, v, fa] = aug_w[t, wa] . aug_f[v, fa]  where
  aug_w = [word_features (80), sentence_output (1)]
  aug_f = [frame_features (64), traj_output (1)]
With TAU = 0.01 every softmax-weighted pooling is within tau*ln(n) <= 0.05 of
a plain max (validated: end-to-end rel err ~1e-4), so the whole fine-grained
path collapses to max-reductions fused onto the matmul's PSUM output.

v2 sharding: the 80 WORDS are split 10-per-core across 8 cores; every core
holds ALL 128 videos. This gives each stationary [d,t] block 8320 moving
columns of reuse (vs 1040 under video-sharding), eliminating ~1000 tiny
8-column matmuls, and makes the cross-core combine an elementwise MAX:
  fw[t,v] = max_cores max_{m in core, fa} S     (AllReduce-max)
  vw[t,v] = max_cores max_{m in core} A         (same AllReduce)
The sentence row (sf = max_fa B, ts = sent.traj) is computed per-core for its
own 16 videos in bf16 and AllGathered (tiny, fully hidden).

The fine sweep runs in fp8 (float8e4) with MatmulPerfMode.DoubleRow: two
128-deep k-chunks per instruction at 2x FLOP rate. fp8 operand noise on a
768-deep dot is ~1.4 absolute on values whose pairwise gaps at the max are
~6+; measured end-to-end loss error is ~1e-4 (tolerance 2e-2).

Per half (64 videos) each core does 10 words x 8 blocks x 3 k-pair matmuls of
[128,512]; the fa-max reduce is split between the Vector engine (direct f32
PSUM reduce) and the Scalar engine (PSUM->SBUF bf16 convert, then a 4x-rate
bf16 DVE reduce) to balance engine load. The half-0 AllReduce(max) overlaps
the half-1 sweep; after the half-1 AllReduce only ~4us of cross-entropy
remains (both ACT tables are pre-warmed during the DMA ramp).
"""

import os
import sys
from contextlib import ExitStack

import numpy as np
import ml_dtypes

_REPO = "/opt/trn_rl_repo"
if os.path.isdir(_REPO) and _REPO not in sys.path:
    sys.path.insert(0, _REPO)

BS, NW, NF, D = 128, 80, 64, 768
N_CORES = 8
MPC = NW // N_CORES          # words per core = 10
NJ = 3                       # k-pairs (768 = 3 * 256)
FINE = BS * NF               # fine moving cols per k-plane = 8192
FREE8 = FINE + BS            # + 128 traj cols = 8320
TAU = 0.01
NEG = -3.0e38

_CACHE = {}


def _build_nc(n_cores=N_CORES):
    """Build + compile the SPMD per-core program (identical on all cores)."""
    from concourse import bacc, mybir, tile

    F32 = mybir.dt.float32
    BF16 = mybir.dt.bfloat16
    FP8 = mybir.dt.float8e4
    AX = mybir.AxisListType.X
    ALU = mybir.AluOpType
    ACT = mybir.ActivationFunctionType
    DR = mybir.MatmulPerfMode.DoubleRow

    nc = bacc.Bacc(
        "TRN2", target_bir_lowering=False, debug=False, num_devices=n_cores
    )
    # DoubleRow operand layouts (i = which 128-deep k-chunk of the 256 pair):
    # weights are plane-major [p, i, cols] (walrus' ldweights ISA check
    # rejects interleave); the MOVING operand is i-interleaved [p, (col, i)]
    # so each column's two k-planes are adjacent bytes (full fetch rate).
    #   wf8 cols: m*128 + t ; ff8 cols: v*64+fa (fine, vl-major) | 8192+v (traj)
    wf8_d = nc.dram_tensor("wf8", [NJ, 128, 2, MPC * 128], FP8, kind="ExternalInput")
    ff8_d = nc.dram_tensor("ff8", [NJ, 128, 2 * FREE8], FP8, kind="ExternalInput")
    sent_d = nc.dram_tensor("sent", [6, 128, 128], BF16, kind="ExternalInput")
    ffv_d = nc.dram_tensor("ffv", [6, 128, 1040], BF16, kind="ExternalInput")
    eye_d = nc.dram_tensor("eye", [128, 128], F32, kind="ExternalInput")
    loss_d = nc.dram_tensor("loss", [1, 1], F32, kind="ExternalOutput")
    sim_d = nc.dram_tensor("sim", [BS, BS], F32, kind="ExternalOutput")

    with tile.TileContext(nc) as tc, ExitStack() as ctx:
        cpool = ctx.enter_context(tc.tile_pool(name="const", bufs=1))
        psA_pool = ctx.enter_context(tc.tile_pool(name="psA", bufs=6, space="PSUM"))
        psb_pool = ctx.enter_context(tc.tile_pool(name="psb", bufs=1, space="PSUM"))
        ps1_pool = ctx.enter_context(tc.tile_pool(name="ps1", bufs=1, space="PSUM"))
        tmp_pool = ctx.enter_context(tc.tile_pool(name="tmp", bufs=3))
        tmpb_pool = ctx.enter_context(tc.tile_pool(name="tmpb", bufs=4))
        dram = ctx.enter_context(tc.tile_pool(name="dram", bufs=1, space="DRAM"))

        # ---- DMA: half-0 j0 operands first so the PE starts ~4us in ------
        ff_sb = [cpool.tile([128, 2 * FREE8], FP8, name=f"ff{j}") for j in range(NJ)]
        wf_sb = [cpool.tile([128, 2, MPC * 128], FP8, name=f"wf{j}") for j in range(NJ)]

        def ff_slice(j, c0, c1):  # cols [c0,c1), both i-planes -> [p, 2, N]
            return ff_sb[j][:, 2 * c0 : 2 * c1].rearrange("p (c i) -> p i c", i=2)

        def ff_dma(j, c0, c1):  # flat byte-range for DMA
            return ff_sb[j][:, 2 * c0 : 2 * c1], ff8_d.ap()[j][:, 2 * c0 : 2 * c1]

        # The first block-pair needs (wf8, ff8 cols 0:1024) for ALL 3 k-pairs;
        # ship exactly that first so the PE starts ~4us in, then the rest of
        # half 0, then the sentence operands, then half 1.
        for j in range(NJ):
            nc.sync.dma_start(*ff_dma(j, 0, 1024))
            nc.sync.dma_start(wf_sb[j][:], wf8_d.ap()[j])
        for j in range(NJ):
            nc.sync.dma_start(*ff_dma(j, 1024, 4096))
            nc.sync.dma_start(*ff_dma(j, FINE, FINE + 64))
        sent_sb = [cpool.tile([128, 128], BF16, name=f"se{k}") for k in range(6)]
        ffv_sb = [cpool.tile([128, 1040], BF16, name=f"fv{k}") for k in range(6)]
        for k in range(6):
            nc.sync.dma_start(sent_sb[k][:], sent_d.ap()[k])
            nc.sync.dma_start(ffv_sb[k][:], ffv_d.ap()[k])
        eye_sb = cpool.tile([128, 128], F32, name="eye_sb")
        nc.gpsimd.dma_start(eye_sb[:], eye_d.ap())
        for j in range(NJ):  # half-1 fine + traj
            nc.sync.dma_start(*ff_dma(j, 4096, FINE))
            nc.sync.dma_start(*ff_dma(j, FINE + 64, FREE8))

        # ---- accumulators + collective buffers ---------------------------
        fwvw = [cpool.tile([128, 128], F32, name=f"fwvw{h}") for h in range(2)]
        nc.vector.memset(fwvw[0][:], NEG)
        nc.vector.memset(fwvw[1][:], NEG)
        s2own = cpool.tile([128, 16], F32, name="s2own")

        ar_in = [dram.tile([128, 128], F32, name=f"ar_in{h}") for h in range(2)]
        ar_out = [
            dram.tile([128, 128], F32, name=f"ar_out{h}", addr_space="Shared")
            for h in range(2)
        ]
        ag_in = dram.tile([128, 16], F32, name="ag_in")
        ag_out = dram.tile([n_cores, 128, 16], F32, name="ag_out", addr_space="Shared")

        # pre-warm both ACT tables (Exp, Ln) so no table load lands in the
        # post-collective tail
        warm = cpool.tile([1, 1], F32, name="warm")
        nc.gpsimd.memset(warm[:], 1.0)
        warm2 = cpool.tile([1, 1], F32, name="warm2")
        nc.scalar.activation(warm2[:], warm[:], ACT.Exp)
        nc.scalar.activation(warm2[:], warm[:], ACT.Ln)

        def wf_ap(j, m):  # stationary [128, 2, 128] for word m, k-pair j
            return wf_sb[j][:, :, m * 128 : (m + 1) * 128]

        DIRECT = (0, 3, 6)  # blocks reduced straight from PSUM on the DVE

        def sentence_sweep():
            # bf16 sentence x (own 16 videos + their traj), AllGather s2.
            # Emitted mid-half-0 so the AG clears the CC stream long before
            # the AllReduces need it.
            psS = [
                psA_pool.tile([128, 512], F32, tag="psA", name=f"psS{q}")
                for q in range(2)
            ]
            psBs = psb_pool.tile([128, 64], F32, tag="psB", name="psBs")
            for k in range(6):
                for q in range(2):
                    nc.tensor.matmul(
                        psS[q][:], lhsT=sent_sb[k][:],
                        rhs=ffv_sb[k][:, q * 512 : (q + 1) * 512],
                        start=(k == 0), stop=(k == 5),
                    )
                nc.tensor.matmul(
                    psBs[:, 0:16], lhsT=sent_sb[k][:], rhs=ffv_sb[k][:, 1024:1040],
                    start=(k == 0), stop=(k == 5),
                )
            for q in range(2):
                nc.vector.reduce_max(
                    s2own[:, 8 * q : 8 * q + 8],
                    psS[q][:].rearrange("p (vl fa) -> p vl fa", vl=8),
                    axis=AX,
                )
            nc.vector.tensor_add(s2own[:], s2own[:], psBs[:, 0:16])
            nc.sync.dma_start(ag_in[:], s2own[:])
            nc.gpsimd.collective_compute(
                "AllGather", ALU.bypass, replica_groups=[list(range(n_cores))],
                ins=[ag_in[:].opt()], outs=[ag_out[:].opt()],
            )

        def fine_half(h):
            c0 = h * 4096
            for m in range(MPC):
                if h == 0 and m == 5:
                    sentence_sweep()
                # bf16 so the block reduces qualify for the DVE 2x/4x modes
                fa_red = tmp_pool.tile([128, 64], BF16, tag="fa_red")
                for pair in range(4):
                    ps = [
                        psA_pool.tile([128, 512], F32, tag="psA", name=f"psA_{q}")
                        for q in range(2)
                    ]
                    for j in range(NJ):
                        for q in range(2):
                            b = 2 * pair + q
                            nc.tensor.matmul(
                                ps[q][:],
                                lhsT=wf_ap(j, m),
                                rhs=ff_slice(j, c0 + b * 512, c0 + (b + 1) * 512),
                                start=(j == 0),
                                stop=(j == NJ - 1),
                                perf_mode=DR,
                            )
                    for q in range(2):
                        b = 2 * pair + q
                        pv = ps[q][:].rearrange("p (vl fa) -> p vl fa", vl=8)
                        if b in DIRECT:
                            nc.vector.reduce_max(fa_red[:, 8 * b : 8 * b + 8], pv, axis=AX)
                        else:
                            sc = tmpb_pool.tile([128, 512], BF16, tag="scb")
                            nc.scalar.activation(sc[:], ps[q][:], ACT.Copy)
                            nc.vector.reduce_max(
                                fa_red[:, 8 * b : 8 * b + 8],
                                sc[:].rearrange("p (vl fa) -> p vl fa", vl=8),
                                axis=AX,
                            )
                # traj cols for this half: A[t, v] partial
                psB = psb_pool.tile([128, 64], F32, tag="psB")
                tc0 = FINE + 64 * h
                for j in range(NJ):
                    nc.tensor.matmul(
                        psB[:],
                        lhsT=wf_ap(j, m),
                        rhs=ff_slice(j, tc0, tc0 + 64),
                        start=(j == 0),
                        stop=(j == NJ - 1),
                        perf_mode=DR,
                    )
                nc.vector.tensor_max(fwvw[h][:, 0:64], fwvw[h][:, 0:64], fa_red[:])
                nc.vector.tensor_max(fwvw[h][:, 64:128], fwvw[h][:, 64:128], psB[:])

        # ---- half 0 sweep, AllReduce(max) #0 ------------------------------
        fine_half(0)
        nc.sync.dma_start(ar_in[0][:], fwvw[0][:])
        nc.gpsimd.collective_compute(
            "AllReduce", ALU.max, replica_groups=[list(range(n_cores))],
            ins=[ar_in[0][:].opt()], outs=[ar_out[0][:].opt()],
        )

        # ---- half 1 sweep, AllReduce(max) #1 ------------------------------
        fine_half(1)
        nc.sync.dma_start(ar_in[1][:], fwvw[1][:])
        nc.gpsimd.collective_compute(
            "AllReduce", ALU.max, replica_groups=[list(range(n_cores))],
            ins=[ar_in[1][:].opt()], outs=[ar_out[1][:].opt()],
        )

        # ---- cross-entropy ------------------------------------------------
        # sim[:, 64h:64h+64] = (fw + vw + s2)/4 ; CE-0 runs under AR#1's
        # shadow, CE-1 + the tiny merge after it.
        s2full = cpool.tile([128, 128], F32, name="s2full")
        nc.sync.dma_start(
            s2full[:].rearrange("p (r c) -> p r c", r=n_cores),
            ag_out[:].rearrange("r p c -> p r c"),
        )
        ones = cpool.tile([128, 1], F32, name="ones")
        nc.gpsimd.memset(ones[:], 1.0)

        mxr = cpool.tile([128, 2], F32, name="mxr")
        nmxr = cpool.tile([128, 2], F32, name="nmxr")
        er = cpool.tile([128, 2], F32, name="er")
        dgh = cpool.tile([128, 2], F32, name="dgh")
        mxc = cpool.tile([64, 2], F32, name="mxc")
        nmxc = cpool.tile([64, 2], F32, name="nmxc")
        ec = cpool.tile([64, 2], F32, name="ec")
        lec = cpool.tile([64, 2], F32, name="lec")
        sL = [cpool.tile([128, 64], F32, name=f"sL{h}") for h in range(2)]

        for h in range(2):
            hh = slice(h, h + 1)
            arf = cpool.tile([128, 128], F32, name=f"arf{h}")
            nc.sync.dma_start(arf[:], ar_out[h][:])
            nc.vector.tensor_add(sL[h][:], arf[:, 0:64], arf[:, 64:128])
            nc.vector.tensor_add(sL[h][:], sL[h][:], s2full[:, 64 * h : 64 * h + 64])
            nc.vector.tensor_scalar_mul(sL[h][:], sL[h][:], 0.25)
            nc.gpsimd.dma_start(sim_d.ap()[:, 64 * h : 64 * h + 64], sL[h][:])

            nc.vector.reduce_max(mxr[:, hh], sL[h][:], axis=AX)
            nc.vector.tensor_scalar_mul(nmxr[:, hh], mxr[:, hh], -1.0)
            scr = tmp_pool.tile([128, 64], F32, tag="scr")
            nc.scalar.activation(
                scr[:], sL[h][:], ACT.Exp,
                bias=nmxr[:, hh], scale=1.0, accum_out=er[:, hh],
            )
            scr2 = tmp_pool.tile([128, 64], F32, tag="scr")
            nc.vector.tensor_mul(scr2[:], sL[h][:], eye_sb[:, 64 * h : 64 * h + 64])
            nc.vector.reduce_sum(dgh[:, hh], scr2[:], axis=AX)
            # column stats via 32x32 DVE block transposes
            sLT = cpool.tile([64, 128], F32, name=f"sLT{h}")
            for bi in range(4):
                for bj in range(2):
                    nc.vector.transpose(
                        sLT[32 * bj : 32 * bj + 32, 32 * bi : 32 * bi + 32],
                        sL[h][32 * bi : 32 * bi + 32, 32 * bj : 32 * bj + 32],
                    )
            nc.vector.reduce_max(mxc[:, hh], sLT[:], axis=AX)
            nc.vector.tensor_scalar_mul(nmxc[:, hh], mxc[:, hh], -1.0)
            scr3 = tmp_pool.tile([64, 128], F32, tag="scrT")
            nc.scalar.activation(
                scr3[:], sLT[:], ACT.Exp,
                bias=nmxc[:, hh], scale=1.0, accum_out=ec[:, hh],
            )

        # merge row stats across halves; assemble loss
        Mrow = cpool.tile([128, 1], F32, name="Mrow")
        nMrow = cpool.tile([128, 1], F32, name="nMrow")
        dsc = cpool.tile([128, 2], F32, name="dsc")
        ew = cpool.tile([128, 2], F32, name="ew")
        es = cpool.tile([128, 1], F32, name="es")
        lser = cpool.tile([128, 1], F32, name="lser")
        nc.vector.tensor_max(Mrow[:], mxr[:, 0:1], mxr[:, 1:2])
        nc.vector.tensor_scalar_mul(nMrow[:], Mrow[:], -1.0)
        nc.scalar.activation(dsc[:], mxr[:], ACT.Exp, bias=nMrow[:], scale=1.0)
        nc.vector.tensor_mul(ew[:], er[:], dsc[:])
        nc.vector.reduce_sum(es[:], ew[:], axis=AX)
        nc.scalar.activation(lec[:], ec[:], ACT.Ln)
        nc.scalar.activation(lser[:], es[:], ACT.Ln)

        dsum = cpool.tile([128, 1], F32, name="dsum")
        rv = cpool.tile([128, 1], F32, name="rv")
        nc.vector.reduce_sum(dsum[:], dgh[:], axis=AX)
        nc.vector.scalar_tensor_tensor(
            out=rv[:], in0=dsum[:], scalar=-2.0, in1=Mrow[:],
            op0=ALU.mult, op1=ALU.add,
        )
        nc.vector.tensor_add(rv[:], rv[:], lser[:])
        cv = cpool.tile([64, 1], F32, name="cv")
        cvb = cpool.tile([64, 1], F32, name="cvb")
        nc.vector.reduce_sum(cv[:], mxc[:], axis=AX)
        nc.vector.reduce_sum(cvb[:], lec[:], axis=AX)
        nc.vector.tensor_add(cv[:], cv[:], cvb[:])

        ps1 = ps1_pool.tile([1, 1], F32, tag="ps1")
        nc.tensor.matmul(ps1[:], lhsT=rv[:], rhs=ones[:], start=True, stop=False)
        nc.tensor.matmul(ps1[:], lhsT=cv[:], rhs=ones[0:64, :], start=False, stop=True)
        lossv = cpool.tile([1, 1], F32, name="lossv")
        nc.vector.tensor_scalar_mul(lossv[:], ps1[:], 1.0 / (2.0 * BS))
        nc.sync.dma_start(loss_d.ap(), lossv[:])

    nc.compile()
    return nc


def _prep_in_maps(wf, ff, so, to, n_cores=N_CORES):
    """Host-side: build per-core operand arrays in matmul layout."""
    fp8 = ml_dtypes.float8_e4m3
    bf = ml_dtypes.bfloat16

    # stationary: wf8[j, p, i, (m, t)] = wf[t, wa=10c+m, d=256j+128i+p]
    W = np.ascontiguousarray(wf.transpose(2, 1, 0)).astype(fp8)   # [d, wa, t]
    W = W.reshape(NJ, 2, 128, NW, BS)                             # [j, i, p, wa, t]

    # moving: ff8[j, p, (col, i)] i-interleaved; col = v*64+fa | 8192+v
    F = np.ascontiguousarray(ff.transpose(2, 0, 1)).astype(fp8)   # [d, v, fa]
    F = F.reshape(NJ, 2, 128, FINE)                               # [j, i, p, v*64+fa]
    T8 = np.ascontiguousarray(to.T).astype(fp8).reshape(NJ, 2, 128, BS)
    FT = np.concatenate([F, T8], axis=3)                          # [j, i, p, 8320]
    ff8 = np.ascontiguousarray(FT.transpose(0, 2, 3, 1)).reshape(NJ, 128, 2 * FREE8)

    # sentence stationary: sent[k, p, t] = so[t, 128k+p]
    sent = np.ascontiguousarray(so.T).astype(bf).reshape(6, 128, 128)

    Fb = np.ascontiguousarray(ff.transpose(2, 0, 1)).astype(bf)   # [d, v, fa]
    Fb = Fb.reshape(6, 128, BS, NF)
    Tb = np.ascontiguousarray(to.T).astype(bf).reshape(6, 128, BS)

    eye = np.eye(128, dtype=np.float32)

    in_maps = []
    for c in range(n_cores):
        wc = W[:, :, :, MPC * c : MPC * (c + 1), :]               # [j,i,p,10,t]
        wf8 = np.ascontiguousarray(wc.transpose(0, 2, 1, 3, 4)).reshape(
            NJ, 128, 2, MPC * 128
        )
        ffv = np.concatenate(
            [
                Fb[:, :, 16 * c : 16 * c + 16, :].reshape(6, 128, 1024),
                Tb[:, :, 16 * c : 16 * c + 16],
            ],
            axis=2,
        )
        in_maps.append(
            {"wf8": wf8, "ff8": ff8, "sent": sent,
             "ffv": np.ascontiguousarray(ffv), "eye": eye}
        )
    return in_maps


def _run(in_maps, trace=False):
    from concourse.bass_utils import run_bass_kernel_spmd

    if "nc" not in _CACHE:
        _CACHE["nc"] = _build_nc()
    return run_bass_kernel_spmd(
        _CACHE["nc"], in_maps, core_ids=list(range(N_CORES)), trace=trace
    )


def _numpy_reference(traj_output, frame_features, sentence_output, word_features,
                     global_mat_weight, word_logit_weight, frame_logit_weight,
                     local_mat_weight, frame_mat_weight, word_mat_weight,
                     frame_mat_weight2, word_mat_weight2):
    """Exact f64 fallback (only used if the weight matrices are not identity)."""
    def softmax(x, axis):
        m = np.max(x, axis=axis, keepdims=True)
        e = np.exp(x - m)
        return e / np.sum(e, axis=axis, keepdims=True)

    def log_softmax(x, axis):
        m = np.max(x, axis=axis, keepdims=True)
        return x - m - np.log(np.sum(np.exp(x - m), axis=axis, keepdims=True))

    to = traj_output.astype(np.float64)
    ff = frame_features.astype(np.float64)
    so = sentence_output.astype(np.float64)
    wf = word_features.astype(np.float64)
    G, WL, FL = (global_mat_weight.astype(np.float64),
                 word_logit_weight.astype(np.float64),
                 frame_logit_weight.astype(np.float64))
    LM, FM, WM = (local_mat_weight.astype(np.float64),
                  frame_mat_weight.astype(np.float64),
                  word_mat_weight.astype(np.float64))
    FM2, WM2 = (frame_mat_weight2.astype(np.float64),
                word_mat_weight2.astype(np.float64))

    traj_sent = (so @ G) @ to.T
    A = np.einsum("twd,vd->twv", wf, to)
    sA = softmax(A / TAU, axis=1)
    wA = np.einsum("twv,wu->tuv", sA, WL)
    video_word = np.sum(A * wA, axis=1)
    B = np.einsum("td,vfd->vtf", so, ff)
    sB = softmax(B / TAU, axis=-1)
    sentence_frame = np.sum(B * (sB @ FL), axis=-1).T
    wfl = wf @ LM
    fw = np.zeros((BS, BS))
    for t in range(BS):
        S = np.einsum("wd,vfd->wvf", wfl[t], ff)
        sw = softmax(S / TAU, axis=0)
        word_level = np.sum(np.einsum("wvf,wu->uvf", sw, WM) * S, axis=0)
        sfx = softmax(S / TAU, axis=-1)
        frame_level = np.sum((sfx @ FM) * S, axis=-1)
        smw = softmax(word_level / TAU, axis=-1)
        s2f = np.sum((smw @ FM2) * word_level, axis=-1)
        smf = softmax(frame_level / TAU, axis=0)
        v2w = np.sum(np.einsum("wv,wu->uv", smf, WM2) * frame_level, axis=0)
        fw[t] = (s2f + v2w) / 2.0
    sim = (traj_sent + video_word + sentence_frame + fw) / 4.0

    def ce(m):
        return -np.mean(np.diagonal(log_softmax(m, -1)))

    return np.array((ce(sim) + ce(sim.T)) / 2.0, dtype=np.float32)


def kernel(**inputs):
    wf = np.ascontiguousarray(np.asarray(inputs["word_features"], np.float32))
    ff = np.ascontiguousarray(np.asarray(inputs["frame_features"], np.float32))
    so = np.ascontiguousarray(np.asarray(inputs["sentence_output"], np.float32))
    to = np.ascontiguousarray(np.asarray(inputs["traj_output"], np.float32))

    eye_names = [
        ("global_mat_weight", D), ("word_logit_weight", NW),
        ("frame_logit_weight", NF), ("local_mat_weight", D),
        ("frame_mat_weight", NF), ("word_mat_weight", NW),
        ("frame_mat_weight2", NF), ("word_mat_weight2", NW),
    ]
    for name, n in eye_names:
        w = np.asarray(inputs[name], np.float32)
        if not np.allclose(w, np.eye(n, dtype=np.float32), atol=1e-6):
            return _numpy_reference(**{k: np.asarray(x) for k, x in inputs.items()})

    res = _run(_prep_in_maps(wf, ff, so, to))
    return np.array(res.results[0]["loss"][0, 0], dtype=np.float32)
